# revision 1
# baseline (speedup 1.0000x reference)
"""Bispectrum kernel for Trainium2 (8 NeuronCores, batch-parallel).

Math: per signal x[L] (L=256), the reference computes
    TM[l1,l2] = (1/L) * sum_n x[n] * x[n-l1] * x[n-l2]   (x zero for idx<0)
averaged over T=8 targets.  Substituting p = L-1-n and
z = [reverse(x)*s, zeros(L)] with s = (L*T)^(-1/3):
    mean_t TM[l1,l2] = sum_t sum_p z[p] * z[p+l1] * z[p+l2]
All three factors come from a Hankel matrix H[p,j] = z[p+j] with
all-positive unit strides (single strided DMA builds it).  On the PE:
    out[l1,l2] += lhsT[p,l1] . rhs[p,l2]
with lhsT[p,l1] = z[p]*z[p+l1] (DVE scale of H by its own column) and
rhs[p,l2] = z[p+l2] (slices of H), accumulating over the two 128-row
K-chunks and the 8 targets directly in PSUM.  The K-chunk p in [128,256)
has identically-zero operands for l1 >= 128, so it only contributes to
the first output row-tile (24 matmuls per batch element instead of 32).
The cube-root prescale makes PSUM hold the final averaged values.
"""

import numpy as np

import concourse.bacc as bacc
import concourse.bass as bass
import concourse.mybir as mybir
import concourse.tile as tile
from concourse.bass_utils import run_bass_kernel_spmd

# Problem shape (hardcoded per contract).
B, T, L = 32, 8, 256
N_CORES = 8
B_CORE = B // N_CORES  # 4 batch elements per core
ZLEN = 2 * L           # 512
W = 384                # hank tile width: j in [0, 384)

_F32R = mybir.dt.float32r
_F32 = mybir.dt.float32


def _build_nc():
    nc = bacc.Bacc("TRN2", target_bir_lowering=False, debug=False)
    z = nc.dram_tensor("z", [B_CORE, T, ZLEN], _F32R, kind="ExternalInput")
    out = nc.dram_tensor("out", [B_CORE, L, L], _F32, kind="ExternalOutput")

    with tile.TileContext(nc) as tc:
        with (
            tc.tile_pool(name="hank", bufs=2) as hank_pool,
            tc.tile_pool(name="a0", bufs=2) as a0_pool,
            tc.tile_pool(name="a1", bufs=2) as a1_pool,
            tc.tile_pool(name="psum", bufs=4, space="PSUM") as psum_pool,
            tc.tile_pool(name="osb", bufs=4) as osb_pool,
        ):
            for b in range(B_CORE):
                hank = hank_pool.tile([128, T, W], _F32R)
                src = bass.AP(
                    tensor=z,
                    offset=b * T * ZLEN,
                    ap=[[1, 128], [ZLEN, T], [1, W]],
                )
                nc.sync.dma_start(out=hank[:], in_=src)

                a0 = a0_pool.tile([128, T, 256], _F32R)
                nc.vector.tensor_tensor(
                    out=a0[:],
                    in0=hank[:, :, 0:256],
                    in1=hank[:, :, 0:1].to_broadcast([128, T, 256]),
                    op=mybir.AluOpType.mult,
                )
                a1 = a1_pool.tile([128, T, 128], _F32R)
                nc.vector.tensor_tensor(
                    out=a1[:],
                    in0=hank[:, :, 128:256],
                    in1=hank[:, :, 128:129].to_broadcast([128, T, 128]),
                    op=mybir.AluOpType.mult,
                )

                for m in range(2):
                    psum = psum_pool.tile([128, 256], _F32)
                    n_acc = 16 if m == 0 else 8
                    i = 0
                    for t in range(T):
                        nc.tensor.matmul(
                            psum[:],
                            lhsT=a0[:, t, m * 128:(m + 1) * 128],
                            rhs=hank[:, t, 0:256],
                            start=(i == 0),
                            stop=(i == n_acc - 1),
                        )
                        i += 1
                    if m == 0:
                        for t in range(T):
                            nc.tensor.matmul(
                                psum[:],
                                lhsT=a1[:, t, :],
                                rhs=hank[:, t, 128:384],
                                start=False,
                                stop=(i == n_acc - 1),
                            )
                            i += 1
                    osb = osb_pool.tile([128, 256], _F32)
                    nc.vector.tensor_copy(out=osb[:], in_=psum[:])
                    nc.sync.dma_start(
                        out=out[b, m * 128:(m + 1) * 128, :], in_=osb[:]
                    )
    nc.finalize()
    return nc


_NC_CACHE = None


def kernel(target: np.ndarray) -> tuple[np.ndarray, np.ndarray]:
    global _NC_CACHE
    target = np.ascontiguousarray(np.asarray(target, dtype=np.float32))
    assert target.shape == (B, T, L), target.shape

    s = np.float32((L * T) ** (-1.0 / 3.0))
    z = np.zeros((B, T, ZLEN), np.float32)
    z[:, :, :L] = target[:, :, ::-1] * s

    if _NC_CACHE is None:
        _NC_CACHE = _build_nc()
    nc = _NC_CACHE

    in_maps = [
        {"z": np.ascontiguousarray(z[i * B_CORE:(i + 1) * B_CORE])}
        for i in range(N_CORES)
    ]
    res = run_bass_kernel_spmd(nc, in_maps, list(range(N_CORES)))
    source = np.concatenate([res.results[i]["out"] for i in range(N_CORES)], axis=0)
    source = source[:, None, :, :]
    return source, target


# revision 2
# speedup vs baseline: 1.0095x; 1.0095x over previous
"""Bispectrum kernel for Trainium2 (8 NeuronCores, batch-parallel).

Math: per signal x[L] (L=256), the reference computes
    TM[l1,l2] = (1/L) * sum_n x[n] * x[n-l1] * x[n-l2]   (x zero for idx<0)
averaged over T=8 targets.  Substituting p = L-1-n and
z = [reverse(x)*s, zeros(L)] with s = (L*T)^(-1/3):
    mean_t TM[l1,l2] = sum_t sum_p z[p] * z[p+l1] * z[p+l2]
All three factors come from a Hankel matrix H[p,j] = z[p+j] with
all-positive unit strides (single strided DMA builds it).  On the PE:
    out[l1,l2] += lhsT[p,l1] . rhs[p,l2]
with lhsT[p,l1] = z[p]*z[p+l1] (DVE scale of H by its own column) and
rhs[p,l2] = z[p+l2] (slices of H), accumulating over the two 128-row
K-chunks and the 8 targets directly in PSUM.  The K-chunk p in [128,256)
has identically-zero operands for l1 >= 128, so it only contributes to
the first output row-tile (24 matmuls per batch element instead of 32).
The cube-root prescale makes PSUM hold the final averaged values.
"""

import numpy as np

import concourse.bacc as bacc
import concourse.bass as bass
import concourse.mybir as mybir
import concourse.tile as tile
from concourse.bass_utils import run_bass_kernel_spmd

# Problem shape (hardcoded per contract).
B, T, L = 32, 8, 256
N_CORES = 8
B_CORE = B // N_CORES  # 4 batch elements per core
ZLEN = 2 * L           # 512
W = 384                # hank tile width: j in [0, 384)

_F32R = mybir.dt.float32r
_F32 = mybir.dt.float32


def _build_nc():
    nc = bacc.Bacc("TRN2", target_bir_lowering=False, debug=False)
    z = nc.dram_tensor("z", [B_CORE, T, ZLEN], _F32R, kind="ExternalInput")
    out = nc.dram_tensor("out", [B_CORE, L, L], _F32, kind="ExternalOutput")

    with tile.TileContext(nc) as tc:
        with (
            tc.tile_pool(name="hank", bufs=2) as hank_pool,
            tc.tile_pool(name="a0", bufs=2) as a0_pool,
            tc.tile_pool(name="a1", bufs=2) as a1_pool,
            tc.tile_pool(name="psum", bufs=4, space="PSUM") as psum_pool,
            tc.tile_pool(name="osb", bufs=4) as osb_pool,
        ):
            for b in range(B_CORE):
                hank = hank_pool.tile([128, T, W], _F32R)
                src = bass.AP(
                    tensor=z,
                    offset=b * T * ZLEN,
                    ap=[[1, 128], [ZLEN, T], [1, W]],
                )
                nc.sync.dma_start(out=hank[:], in_=src)

                a0 = a0_pool.tile([128, T, 256], _F32R)
                nc.vector.tensor_tensor(
                    out=a0[:],
                    in0=hank[:, :, 0:256],
                    in1=hank[:, :, 0:1].to_broadcast([128, T, 256]),
                    op=mybir.AluOpType.mult,
                )
                a1 = a1_pool.tile([128, T, 128], _F32R)
                nc.vector.tensor_tensor(
                    out=a1[:],
                    in0=hank[:, :, 128:256],
                    in1=hank[:, :, 128:129].to_broadcast([128, T, 128]),
                    op=mybir.AluOpType.mult,
                )

                for m in range(2):
                    psum = psum_pool.tile([128, 256], _F32)
                    n_acc = 16 if m == 0 else 8
                    i = 0
                    for t in range(T):
                        nc.tensor.matmul(
                            psum[:],
                            lhsT=a0[:, t, m * 128:(m + 1) * 128],
                            rhs=hank[:, t, 0:256],
                            start=(i == 0),
                            stop=(i == n_acc - 1),
                        )
                        i += 1
                    if m == 0:
                        for t in range(T):
                            nc.tensor.matmul(
                                psum[:],
                                lhsT=a1[:, t, :],
                                rhs=hank[:, t, 128:384],
                                start=False,
                                stop=(i == n_acc - 1),
                            )
                            i += 1
                    osb = osb_pool.tile([128, 256], _F32)
                    nc.vector.tensor_copy(out=osb[:], in_=psum[:])
                    nc.sync.dma_start(
                        out=out[b, m * 128:(m + 1) * 128, :], in_=osb[:]
                    )
    nc.finalize()
    return nc


_NC_CACHE = None


def get_nc():
    global _NC_CACHE
    if _NC_CACHE is None:
        _NC_CACHE = _build_nc()
    return _NC_CACHE


def prepare_in_maps(target: np.ndarray):
    """Host-side prep: reversed/prescaled/zero-padded z, sharded by batch.

    Returns (in_maps, assemble) where assemble(results) -> source.
    """
    target = np.ascontiguousarray(np.asarray(target, dtype=np.float32))
    assert target.shape == (B, T, L), target.shape
    s = np.float32((L * T) ** (-1.0 / 3.0))
    z = np.zeros((B, T, ZLEN), np.float32)
    z[:, :, :L] = target[:, :, ::-1] * s
    in_maps = [
        {"z": np.ascontiguousarray(z[i * B_CORE:(i + 1) * B_CORE])}
        for i in range(N_CORES)
    ]

    def assemble(results):
        source = np.concatenate(
            [results[i]["out"] for i in range(N_CORES)], axis=0
        )
        return source[:, None, :, :]

    return in_maps, assemble


def kernel(target: np.ndarray) -> tuple[np.ndarray, np.ndarray]:
    target = np.ascontiguousarray(np.asarray(target, dtype=np.float32))
    in_maps, assemble = prepare_in_maps(target)
    res = run_bass_kernel_spmd(get_nc(), in_maps, list(range(N_CORES)))
    return assemble(res.results), target


# revision 7
# speedup vs baseline: 1.3895x; 1.3765x over previous
"""Bispectrum kernel for Trainium2 (8 NeuronCores, batch-parallel).

Math: per signal x[L] (L=256), the reference computes
    TM[l1,l2] = (1/L) * sum_n x[n] * x[n-l1] * x[n-l2]   (x zero for idx<0)
averaged over T=8 targets.  Substituting p = L-1-n and
z = [reverse(x)*s, zeros(L)] with s = (L*T)^(-1/3):
    mean_t TM[l1,l2] = sum_t sum_p z[p] * z[p+l1] * z[p+l2]
All three factors come from a Hankel matrix H[p,j] = z[p+j] with
all-positive unit strides (single strided DMA builds it).  On the PE:
    out[l1,l2] += lhsT[p,l1] . rhs[p,l2]
with lhsT[p,l1] = z[p]*z[p+l1] (DVE scale of H by its own column) and
rhs[p,l2] = z[p+l2] (slices of H), accumulating over the two 128-row
K-chunks and the 8 targets directly in PSUM.  The K-chunk p in [128,256)
has identically-zero operands for l1 >= 128, so it only contributes to
the first output row-tile (24 matmuls per batch element instead of 32).
The cube-root prescale makes PSUM hold the final averaged values.
"""

import numpy as np

import concourse.bacc as bacc
import concourse.bass as bass
import concourse.mybir as mybir
import concourse.tile as tile
from concourse.bass_utils import run_bass_kernel_spmd

# Problem shape (hardcoded per contract).
B, T, L = 32, 8, 256
N_CORES = 8
B_CORE = B // N_CORES  # 4 batch elements per core
ZLEN = 2 * L           # 512
W = 384                # hank tile width: j in [0, 384)

_F32R = mybir.dt.float32r
_F32 = mybir.dt.float32


def _build_nc():
    nc = bacc.Bacc("TRN2", target_bir_lowering=False, debug=False)
    z = nc.dram_tensor("z", [B_CORE, T, ZLEN], _F32R, kind="ExternalInput")
    out = nc.dram_tensor("out", [B_CORE, L, L], _F32, kind="ExternalOutput")

    with tile.TileContext(nc) as tc:
        with (
            tc.tile_pool(name="hank", bufs=1) as hank_pool,
            tc.tile_pool(name="a0", bufs=2) as a0_pool,
            tc.tile_pool(name="a1", bufs=2) as a1_pool,
            tc.tile_pool(name="psum", bufs=4, space="PSUM") as psum_pool,
            tc.tile_pool(name="osb", bufs=2) as osb_pool,
        ):
            # One hank buffer per batch element.  The DMA fills j in [0,256);
            # j in [256,384) is z's zero tail (indices >= 256+p), memset once
            # on the otherwise-idle GpSimd engine.
            zcol = hank_pool.tile([128, 1], _F32, tag="zcol")
            nc.vector.memset(zcol[:], 0.0)
            hanks = []
            for b in range(B_CORE):
                hank = hank_pool.tile([128, T, W], _F32R, tag=f"hank{b}")
                nc.vector.tensor_copy(
                    out=hank[:, :, 256:W],
                    in_=zcol[:, 0:1].to_broadcast([128, T, W - 256]),
                )
                src = bass.AP(
                    tensor=z,
                    offset=b * T * ZLEN,
                    ap=[[1, 128], [ZLEN, T], [1, 256]],
                )
                nc.sync.dma_start(out=hank[:, :, 0:256], in_=src)
                hanks.append(hank)

            for b in range(B_CORE):
                hank = hanks[b]
                a0 = a0_pool.tile([128, T, 256], _F32R)
                nc.vector.tensor_tensor(
                    out=a0[:],
                    in0=hank[:, :, 0:256],
                    in1=hank[:, :, 0:1].to_broadcast([128, T, 256]),
                    op=mybir.AluOpType.mult,
                )
                a1 = a1_pool.tile([128, T, 128], _F32R)
                nc.vector.tensor_tensor(
                    out=a1[:],
                    in0=hank[:, :, 128:256],
                    in1=hank[:, :, 128:129].to_broadcast([128, T, 128]),
                    op=mybir.AluOpType.mult,
                )

                osb = osb_pool.tile([128, 2, 256], _F32)
                for m in range(2):
                    psum = psum_pool.tile([128, 256], _F32)
                    n_acc = 16 if m == 0 else 8
                    i = 0
                    for t in range(T):
                        nc.tensor.matmul(
                            psum[:],
                            lhsT=a0[:, t, m * 128:(m + 1) * 128],
                            rhs=hank[:, t, 0:256],
                            start=(i == 0),
                            stop=(i == n_acc - 1),
                        )
                        i += 1
                    if m == 0:
                        for t in range(T):
                            nc.tensor.matmul(
                                psum[:],
                                lhsT=a1[:, t, :],
                                rhs=hank[:, t, 128:384],
                                start=False,
                                stop=(i == n_acc - 1),
                            )
                            i += 1
                    nc.vector.tensor_copy(out=osb[:, m, :], in_=psum[:])
                # One 256KB DMA per batch: dest[b, m*128+p, l] = osb[p, m, l].
                dst = bass.AP(
                    tensor=out,
                    offset=b * L * L,
                    ap=[[L, 128], [128 * L, 2], [1, L]],
                )
                nc.sync.dma_start(out=dst, in_=osb[:])
    nc.finalize()
    return nc


_NC_CACHE = None


def get_nc():
    global _NC_CACHE
    if _NC_CACHE is None:
        _NC_CACHE = _build_nc()
    return _NC_CACHE


def prepare_in_maps(target: np.ndarray):
    """Host-side prep: reversed/prescaled/zero-padded z, sharded by batch.

    Returns (in_maps, assemble) where assemble(results) -> source.
    """
    target = np.ascontiguousarray(np.asarray(target, dtype=np.float32))
    assert target.shape == (B, T, L), target.shape
    s = np.float32((L * T) ** (-1.0 / 3.0))
    z = np.zeros((B, T, ZLEN), np.float32)
    z[:, :, :L] = target[:, :, ::-1] * s
    in_maps = [
        {"z": np.ascontiguousarray(z[i * B_CORE:(i + 1) * B_CORE])}
        for i in range(N_CORES)
    ]

    def assemble(results):
        source = np.concatenate(
            [results[i]["out"] for i in range(N_CORES)], axis=0
        )
        return source[:, None, :, :]

    return in_maps, assemble


def kernel(target: np.ndarray) -> tuple[np.ndarray, np.ndarray]:
    target = np.ascontiguousarray(np.asarray(target, dtype=np.float32))
    in_maps, assemble = prepare_in_maps(target)
    res = run_bass_kernel_spmd(get_nc(), in_maps, list(range(N_CORES)))
    return assemble(res.results), target
